# revision 20
# baseline (speedup 1.0000x reference)
"""Trainium2 Bass kernel for nn_EnvironmentSpecificDecoder.

Data-parallel over batch B=32 across 8 NeuronCores (NB=4 batches/core).

All matmuls in bf16 (1 cyc/row on PE at any N, FWL weight loads, half the
input DMA bytes vs fp32; end-to-end relerr ~5e-3 vs the 2e-2 gate).

Layout trick: pairs hold (t, t+2) so that all per-oct intermediates are
contiguous in "t-ascending" order and no strided matmul operands appear.
Per batch b, oct o (8 t's = 2 quads):
  stage1: 4 MMs  p1[(t01,l),(qq,tp,i)] = zz_pair^T @ A          (N=128)
  S23   : 4 MMs row-packed par=t01 pairs (concurrent K=64 tiles):
          p23[h,(qq,tq,i)] = W1s^T zzt  with W1s = W_sig@W1[env]
          (env dispatched per batch by regime via dynamic-offset DMA)
  C1    : 4 MMs row-packed: pc[h2,(tq,i)] = Wc^T zcT (zcT host-transposed)
  S4+C2 : per quad 3 accumulating MMs (W2 halves + Wo) into one PSUM bank,
          col-packed 4 quads/bank at partition bases {0,32,64,96}
  evac  : relu+bias fused PSUM->SBUF casts split across Scalar/Vector
  tail  : one [128,512] bias ACT per 2 octs; mu rows DMA straight to DRAM,
          sigma rows to a dense tile for one softplus pass at the end.
"""
import numpy as np
import ml_dtypes

N_CORES = 8
NB = 4          # batches per core
T = 64
D = 128
L = 64
H = 256
H2 = 128
NE = 8

_CACHE = {}


def _round_fp32r(x: np.ndarray) -> np.ndarray:
    """Round fp32 array to E8M11 (float32r) with round-to-nearest-even."""
    u = np.ascontiguousarray(x, dtype=np.float32).view(np.uint32)
    keep = np.uint32(12)
    half = np.uint32(1 << 11)
    lsb = (u >> keep) & np.uint32(1)
    return ((u + (half - np.uint32(1) + lsb)) >> keep << keep).view(np.float32)


def _build():
    import concourse.bacc as bacc
    import concourse.bass as bass
    import concourse.mybir as mybir
    from concourse.tile import TileContext

    F32 = mybir.dt.float32
    F32R = mybir.dt.float32r
    BF16 = mybir.dt.bfloat16
    AF = mybir.ActivationFunctionType
    ADD = mybir.AluOpType.add
    MAX = mybir.AluOpType.max

    nc = bacc.Bacc("TRN2", target_bir_lowering=False, debug=False)

    # inputs (host pre-packed, see _prepare_in_maps)
    zs_d = nc.dram_tensor("zs", [NB, D, T * L], BF16, kind="ExternalInput")
    zc_d = nc.dram_tensor("zc", [NB, D, T * L], BF16, kind="ExternalInput")
    ai_d = nc.dram_tensor("ai", [D, D], BF16, kind="ExternalInput")
    reg_d = nc.dram_tensor("reg", [1, NB], mybir.dt.int32, kind="ExternalInput")
    w1s_d = nc.dram_tensor("w1s", [NE, D, H], F32R, kind="ExternalInput")
    b1s_d = nc.dram_tensor("b1s", [NE, D, 2], F32, kind="ExternalInput")
    w2p_d = nc.dram_tensor("w2p", [NE, D, 2, 32], BF16, kind="ExternalInput")
    b2b_d = nc.dram_tensor("b2b", [NE, D, 1], F32, kind="ExternalInput")
    wc_d = nc.dram_tensor("wc", [D, 2 * H2], BF16, kind="ExternalInput")
    bc_d = nc.dram_tensor("bc", [H2, 1], F32, kind="ExternalInput")
    wo_d = nc.dram_tensor("wo", [H2, 32], BF16, kind="ExternalInput")

    mu_d = nc.dram_tensor("mu", [NB, T * D], F32, kind="ExternalOutput")
    sg_d = nc.dram_tensor("sg", [NB, T * D], F32, kind="ExternalOutput")

    with TileContext(nc) as tc:
        with (
            tc.tile_pool(name="const", bufs=1) as constp,
            tc.tile_pool(name="zin", bufs=4) as zinp,
            tc.tile_pool(name="ev", bufs=2) as evp,
            tc.tile_pool(name="stg", bufs=3) as stgp,
            tc.tile_pool(name="fin", bufs=1) as finp,
            tc.tile_pool(name="ps1", bufs=1, space="PSUM") as ps1,
            tc.tile_pool(name="ps23", bufs=1, space="PSUM") as ps23,
            tc.tile_pool(name="psc", bufs=2, space="PSUM") as psc,
            tc.tile_pool(name="ps4", bufs=1, space="PSUM") as ps4,
        ):
            # ---- static weights (reg first: it gates the dispatch) ----
            reg_sb = constp.tile([1, NB], mybir.dt.int32)
            nc.sync.dma_start(reg_sb[:], reg_d[:])
            ai_sb = constp.tile([D, D], BF16)
            nc.sync.dma_start(ai_sb[:], ai_d[:])
            wc_sb = constp.tile([D, 2 * H2], BF16)   # [[Wc;0] | [0;Wc]]
            nc.sync.dma_start(wc_sb[:], wc_d[:])
            wo_sb = constp.tile([H2, 32], BF16)
            nc.sync.dma_start(wo_sb[:], wo_d[:])
            bc_sb = constp.tile([H2, 1], F32)
            nc.sync.dma_start(bc_sb[:], bc_d[:])

            # ---- batch-0 first chunks via the idle ACT ring: fast start
            zz_t, zc_t = [], []
            for b in range(NB):
                zz_t.append(zinp.tile([D, T * L], BF16, tag="zz",
                                      name=f"zz{b}"))
                zc_t.append(zinp.tile([D, T * L], BF16, tag="zc",
                                      name=f"zc{b}"))
            nc.scalar.dma_start(zz_t[0][:, 0:2048], zs_d[0, :, 0:2048])
            nc.scalar.dma_start(zc_t[0][:, 0:2048], zc_d[0, :, 0:2048])

            # ---- per-batch dispatched weights (regime -> env) ----
            w1s_sb, b1s_sb, w2_sb, b2b_sb = [], [], [], []
            for b in range(NB):
                e = nc.values_load(
                    reg_sb[0:1, b : b + 1],
                    engines=[mybir.EngineType.SP],
                    min_val=0, max_val=NE - 1,
                    skip_runtime_bounds_check=True,
                )
                w1 = constp.tile([D, H], F32R, name=f"w1s{b}", tag=f"w1s{b}")
                nc.sync.dma_start(
                    w1[:], w1s_d[bass.ds(e, 1)].rearrange("o p h -> (o p) h")
                )
                b1 = constp.tile([D, 2], F32, name=f"b1s{b}", tag=f"b1s{b}")
                nc.sync.dma_start(
                    b1[:], b1s_d[bass.ds(e, 1)].rearrange("o p h -> (o p) h")
                )
                w2 = constp.tile([D, 2, 32], BF16, name=f"w2{b}", tag=f"w2{b}")
                nc.sync.dma_start(
                    w2[:], w2p_d[bass.ds(e, 1)].rearrange("o p a k -> (o p) a k")
                )
                b2 = constp.tile([D, 1], F32, name=f"b2b{b}", tag=f"b2b{b}")
                nc.sync.dma_start(
                    b2[:], b2b_d[bass.ds(e, 1)].rearrange("o p h -> (o p) h")
                )
                w1s_sb.append(w1)
                b1s_sb.append(b1)
                w2_sb.append(w2)
                b2b_sb.append(b2)

            # ---- remaining input chunks: issued lazily on the ACT ring
            # (one per odd oct) so they never head-of-line-block outputs
            pending = [(zz_t[0][:, 2048:4096], zs_d[0, :, 2048:4096]),
                       (zc_t[0][:, 2048:4096], zc_d[0, :, 2048:4096])]
            for b in range(1, NB):
                for ch in range(2):
                    cs = 2048 * ch
                    pending.append((zz_t[b][:, cs : cs + 2048],
                                    zs_d[b, :, cs : cs + 2048]))
                    pending.append((zc_t[b][:, cs : cs + 2048],
                                    zc_d[b, :, cs : cs + 2048]))

            st_sig = finp.tile([NB * 32, 512], F32)

            # preload the Ln activation table so the per-batch softplus
            # doesn't pay a 1.3us ACT_TABLE_LOAD on the critical tail
            warm = finp.tile([1, 1], F32)
            nc.scalar.activation(warm[:], bc_sb[0:1, 0:1], AF.Ln, bias=1.0)
            nc.scalar.activation(warm[:], warm[:], AF.Exp)

            for b in range(NB):
                zz = zz_t[b]
                zc = zc_t[b]

                for o in range(8):
                    if o % 2 == 1:
                        n_pop = 2 if (b == 0 and o == 1) else 1
                        for _ in range(n_pop):
                            if pending:
                                dst_ap, src_ap = pending.pop(0)
                                nc.scalar.dma_start(dst_ap, src_ap)
                    # ---- stage 1: 4 signal pair matmuls, N=128 ----
                    p1 = ps1.tile([D, 512], F32, tag="p1")
                    for qt in range(4):           # qt = qq*2+tp
                        pr = o * 4 + qt
                        nc.tensor.matmul(
                            p1[:, 128 * qt : 128 * (qt + 1)],
                            zz[:, 128 * pr : 128 * (pr + 1)],
                            ai_sb[:],
                            start=True, stop=True,
                        )
                    # ---- C1 (independent of zzt; hides the cast) ----
                    # pc_q[h2, (tq,i)] per qq; par tiles run concurrently
                    pcs = []
                    for qq in range(2):
                        pc = psc.tile([D, 512], F32, tag="pc")
                        for par in range(2):
                            nc.tensor.matmul(
                                pc[:, 256 * par : 256 * (par + 1)],
                                wc_sb[:, 128 * par : 128 * (par + 1)],
                                zc[:, 512 * o + 256 * qq :
                                   512 * o + 256 * qq + 256],
                                start=True, stop=True,
                            )
                        pcs.append(pc)

                    # ---- stage-1 evacuation: fp32 PSUM -> bf16 SBUF ----
                    zzt = evp.tile([D, 512], F32R, tag="zzt")
                    nc.vector.tensor_copy(zzt[:], p1[:])


                    # ---- S23 (hh-major) + h1 evac right after each hh ----
                    # h1 cols: hh*1024 + qq*512 + tq*128 + i  (tq = 2*par+tp)
                    h1 = evp.tile([D, 2048], BF16, tag="h1")
                    h1v = h1[:].rearrange(
                        "p (hh qq par c) -> p hh qq par c", hh=2, qq=2, par=2)
                    for hh in range(2):
                        ph = ps23.tile([D, 1024], F32, tag=f"p23h{hh}")
                        for par in range(2):
                            nc.tensor.matmul(
                                ph[:, 512 * par : 512 * par + 512],
                                w1s_sb[b][64 * par : 64 * par + 64,
                                          128 * hh : 128 * (hh + 1)],
                                zzt[64 * par : 64 * par + 64, :],
                                start=True, stop=True,
                            )
                        in_ap = ph[:].rearrange(
                            "p (par qq c) -> p par qq c", par=2, qq=2
                        ).transpose([0, 2, 1, 3])
                        out_ap = h1v[:, hh]
                        if hh == 0:
                            nc.scalar.activation(
                                out_ap, in_ap, AF.Relu,
                                bias=b1s_sb[b][:, 0:1],
                            )
                        else:
                            nc.vector.tensor_scalar(
                                out_ap, in_ap,
                                b1s_sb[b][:, 1:2], 0.0, ADD, MAX,
                            )


                    # ---- hc evac: relu(pc + bc) -> bf16 ----
                    hcs = evp.tile([D, 1024], BF16, tag="hcs")
                    nc.scalar.activation(
                        hcs[:, 0:512], pcs[0][:], AF.Relu, bias=bc_sb[:, 0:1])
                    nc.scalar.activation(
                        hcs[:, 512:768], pcs[1][:, 0:256], AF.Relu,
                        bias=bc_sb[:, 0:1])
                    nc.vector.tensor_scalar(
                        hcs[:, 768:1024], pcs[1][:, 256:512],
                        bc_sb[:, 0:1], 0.0, ADD, MAX)

                    # ---- S4 + C2: col-packed quads, 2 per PSUM bank ----
                    p4 = ps4.tile([D, 512], F32, tag="p4")
                    for qq in range(2):
                        bp = qq * 32
                        nc.tensor.matmul(
                            p4[bp : bp + 32, :], w2_sb[b][:, 0, :],
                            h1[:, 512 * qq : 512 * qq + 512],
                            start=True, stop=False,
                        )
                        nc.tensor.matmul(
                            p4[bp : bp + 32, :], w2_sb[b][:, 1, :],
                            h1[:, 1024 + 512 * qq : 1024 + 512 * qq + 512],
                            start=False, stop=False,
                        )
                        nc.tensor.matmul(
                            p4[bp : bp + 32, :], wo_sb[:],
                            hcs[:, 512 * qq : 512 * qq + 512],
                            start=False, stop=True,
                        )
                    # one bias pass covers both quads (rows 0,1,32,33)
                    if o % 2 == 0:
                        stb2 = stgp.tile([64, 1024], F32, tag="stb")
                    nc.scalar.activation(
                        stb2[:, 512 * (o % 2) : 512 * (o % 2) + 512],
                        p4[0:64, :], AF.Identity, bias=b2b_sb[b][0:64, 0:1])
                    if o % 2 == 1:
                        o0 = o - 1
                        for q in range(2):
                            off = 1024 * o0 + 512 * q
                            nc.sync.dma_start(
                                mu_d[b : b + 1, off : off + 1536]
                                .rearrange("z (h c) -> z h c", h=3)[:, 0::2],
                                stb2[32 * q : 32 * q + 1, :],
                            )
                            r0 = b * 32 + 8 * q + o0
                            nc.sync.dma_start(
                                st_sig[r0 : r0 + 2, :],
                                stb2[32 * q + 1 : 32 * q + 2, :],
                            )

                # ---- sigma tail for this batch: softplus + 0.01 + out ----
                sgrows = st_sig[b * 32 : b * 32 + 16, :]
                ex = stgp.tile([16, 512], F32, tag="ex")
                nc.scalar.activation(ex[:], sgrows, AF.Exp)
                nc.scalar.activation(sgrows, ex[:], AF.Ln, bias=1.0)
                nc.vector.tensor_scalar_add(sgrows, sgrows, 0.01)
                for q in range(2):
                    nc.sync.dma_start(
                        sg_d[b : b + 1, :]
                        .rearrange("z (o q c) -> z o q c", o=8, q=2)[:, :, q],
                        st_sig[b * 32 + 8 * q : b * 32 + 8 * q + 8, :],
                    )



    nc.compile()
    return nc


def _get_nc():
    if "nc" not in _CACHE:
        _CACHE["nc"] = _build()
    return _CACHE["nc"]


def _prepare_in_maps(z_signal, z_corrupt, A, regime, W_sig, b_sig, W1e, b1e,
                     W2e, b2e, Wc, bc, Wo, bo):
    bf16 = ml_dtypes.bfloat16
    z_signal = np.asarray(z_signal, dtype=np.float32)
    z_corrupt = np.asarray(z_corrupt, dtype=np.float32)
    A = np.asarray(A, dtype=np.float32)
    regime = np.asarray(regime)
    W_sig = np.asarray(W_sig, dtype=np.float32)
    b_sig = np.asarray(b_sig, dtype=np.float32)
    W1e = np.asarray(W1e, dtype=np.float32)
    b1e = np.asarray(b1e, dtype=np.float32)
    W2e = np.asarray(W2e, dtype=np.float32)
    b2e = np.asarray(b2e, dtype=np.float32)
    Wc = np.asarray(Wc, dtype=np.float32)
    bc = np.asarray(bc, dtype=np.float32)
    Wo = np.asarray(Wo, dtype=np.float32)
    bo = np.asarray(bo, dtype=np.float32)

    eidx = np.where(regime >= NE, 0, regime).astype(np.int32)

    # ---- host weight transforms (env tables, replicated to all cores) ----
    ai = A.astype(bf16)
    w1s_half = np.einsum("lh,ehk->elk", W_sig, W1e)            # [E, L, H]
    w1s = _round_fp32r(
        np.ascontiguousarray(np.concatenate([w1s_half, w1s_half], axis=1)))
    b1s_full = np.einsum("h,ehk->ek", b_sig, W1e) + b1e        # [E, H]
    b1s = np.ascontiguousarray(
        b1s_full.reshape(NE, 2, D).transpose(0, 2, 1))         # [E, D, 2]
    w2p = np.zeros((NE, D, 2, 32), np.float32)
    w2p[..., 0:2] = W2e.reshape(NE, 2, D, 2).transpose(0, 2, 1, 3)
    w2p = w2p.astype(bf16)
    b2b = np.zeros((NE, D, 1), np.float32)
    b2b[:, 0::32, 0] = (b2e[:, 0] + bo[0])[:, None]
    b2b[:, 1::32, 0] = b2e[:, 1][:, None]
    wc_r = np.zeros((D, 2 * H2), np.float32)                   # [[Wc;0]|[0;Wc]]
    wc_r[0:64, 0:H2] = Wc
    wc_r[64:128, H2:] = Wc
    wc_r = wc_r.astype(bf16)
    wo_r = np.zeros((H2, 32), np.float32)
    wo_r[:, 0:1] = Wo
    wo_r = wo_r.astype(bf16)
    bc_r = np.ascontiguousarray(bc[:, None])                   # [H2, 1]

    in_maps = []
    for c in range(N_CORES):
        b0 = c * NB
        zs4 = z_signal[b0 : b0 + NB]
        zc4 = z_corrupt[b0 : b0 + NB]
        # signal: [nb, D, (o,qq,tp), (t01,l)] — pair pr holds (t, t+2)
        zt = zs4.transpose(0, 2, 1, 3).reshape(NB, D, 8, 2, 2, 2, L)
        zs_p = np.ascontiguousarray(
            zt.transpose(0, 1, 2, 3, 5, 4, 6).reshape(NB, D, T * L)
        ).astype(bf16)
        # corrupt (host-transposed): [nb, (t01,l), (o,qq,tp,i)]
        zcr = zc4.reshape(NB, 8, 2, 2, 2, D, L)
        zc_p = np.ascontiguousarray(
            zcr.transpose(0, 3, 6, 1, 2, 4, 5).reshape(NB, D, T * L)
        ).astype(bf16)
        in_maps.append({
            "zs": zs_p,
            "zc": zc_p,
            "ai": ai,
            "reg": eidx[None, b0 : b0 + NB],
            "w1s": w1s,
            "b1s": b1s,
            "w2p": w2p,
            "b2b": b2b,
            "wc": wc_r,
            "bc": bc_r,
            "wo": wo_r,
        })
    return in_maps


def kernel(z_signal, z_corrupt, A, regime, W_sig, b_sig, W1e, b1e, W2e, b2e,
           Wc, bc, Wo, bo):
    from concourse.bass_utils import run_bass_kernel_spmd

    in_maps = _prepare_in_maps(z_signal, z_corrupt, A, regime, W_sig, b_sig,
                               W1e, b1e, W2e, b2e, Wc, bc, Wo, bo)
    nc = _get_nc()
    res = run_bass_kernel_spmd(nc, in_maps, core_ids=list(range(N_CORES)))

    mu = np.concatenate(
        [r["mu"].reshape(NB, T, D) for r in res.results], axis=0)
    sigma = np.concatenate(
        [r["sg"].reshape(NB, T, D) for r in res.results], axis=0)
    return mu, sigma


def run_traced(inputs_np):
    from concourse.bass_utils import run_bass_kernel_spmd

    in_maps = _prepare_in_maps(**inputs_np)
    nc = _get_nc()
    return run_bass_kernel_spmd(
        nc, in_maps, core_ids=list(range(N_CORES)), trace=True
    )


# revision 21
# speedup vs baseline: 1.1117x; 1.1117x over previous
"""Trainium2 Bass kernel for nn_EnvironmentSpecificDecoder.

Data-parallel over batch B=32 across 8 NeuronCores (NB=4 batches/core).

All matmuls in bf16 (1 cyc/row on PE at any N, FWL weight loads, half the
input DMA bytes vs fp32; end-to-end relerr ~5e-3 vs the 2e-2 gate).

Layout trick: pairs hold (t, t+2) so that all per-oct intermediates are
contiguous in "t-ascending" order and no strided matmul operands appear.
Per batch b, oct o (8 t's = 2 quads):
  stage1: 4 MMs  p1[(t01,l),(qq,tp,i)] = zz_pair^T @ A          (N=128)
  S23   : 4 MMs row-packed par=t01 pairs (concurrent K=64 tiles):
          p23[h,(qq,tq,i)] = W1s^T zzt  with W1s = W_sig@W1[env]
          (env dispatched per batch by regime via dynamic-offset DMA)
  C1    : 4 MMs row-packed: pc[h2,(tq,i)] = Wc^T zcT (zcT host-transposed)
  S4+C2 : per quad 3 accumulating MMs (W2 halves + Wo) into one PSUM bank,
          col-packed 4 quads/bank at partition bases {0,32,64,96}
  evac  : relu+bias fused PSUM->SBUF casts split across Scalar/Vector
  tail  : one [128,512] bias ACT per 2 octs; mu rows DMA straight to DRAM,
          sigma rows to a dense tile for one softplus pass at the end.
"""
import numpy as np
import ml_dtypes

N_CORES = 8
NB = 4          # batches per core
T = 64
D = 128
L = 64
H = 256
H2 = 128
NE = 8

_CACHE = {}


def _round_fp32r(x: np.ndarray) -> np.ndarray:
    """Round fp32 array to E8M11 (float32r) with round-to-nearest-even."""
    u = np.ascontiguousarray(x, dtype=np.float32).view(np.uint32)
    keep = np.uint32(12)
    half = np.uint32(1 << 11)
    lsb = (u >> keep) & np.uint32(1)
    return ((u + (half - np.uint32(1) + lsb)) >> keep << keep).view(np.float32)


def _build():
    import concourse.bacc as bacc
    import concourse.bass as bass
    import concourse.mybir as mybir
    from concourse.tile import TileContext

    F32 = mybir.dt.float32
    F32R = mybir.dt.float32r
    BF16 = mybir.dt.bfloat16
    AF = mybir.ActivationFunctionType
    ADD = mybir.AluOpType.add
    MAX = mybir.AluOpType.max

    nc = bacc.Bacc("TRN2", target_bir_lowering=False, debug=False)

    # inputs (host pre-packed, see _prepare_in_maps)
    zs_d = nc.dram_tensor("zs", [NB, D, T * L], BF16, kind="ExternalInput")
    zc_d = nc.dram_tensor("zc", [NB, D, T * L], BF16, kind="ExternalInput")
    ai_d = nc.dram_tensor("ai", [D, D], BF16, kind="ExternalInput")
    reg_d = nc.dram_tensor("reg", [1, NB], mybir.dt.int32, kind="ExternalInput")
    w1s_d = nc.dram_tensor("w1s", [NE, D, H], F32R, kind="ExternalInput")
    b1s_d = nc.dram_tensor("b1s", [NE, D, 2], F32, kind="ExternalInput")
    w2p_d = nc.dram_tensor("w2p", [NE, D, 2, 32], BF16, kind="ExternalInput")
    b2b_d = nc.dram_tensor("b2b", [NE, D, 1], F32, kind="ExternalInput")
    wc_d = nc.dram_tensor("wc", [D, 2 * H2], BF16, kind="ExternalInput")
    bc_d = nc.dram_tensor("bc", [H2, 1], F32, kind="ExternalInput")
    wo_d = nc.dram_tensor("wo", [H2, 32], BF16, kind="ExternalInput")

    mu_d = nc.dram_tensor("mu", [NB, T * D], F32, kind="ExternalOutput")
    sg_d = nc.dram_tensor("sg", [NB, T * D], F32, kind="ExternalOutput")

    with TileContext(nc) as tc:
        with (
            tc.tile_pool(name="const", bufs=1) as constp,
            tc.tile_pool(name="zin", bufs=4) as zinp,
            tc.tile_pool(name="ev", bufs=2) as evp,
            tc.tile_pool(name="stg", bufs=3) as stgp,
            tc.tile_pool(name="fin", bufs=1) as finp,
            tc.tile_pool(name="ps1", bufs=1, space="PSUM") as ps1,
            tc.tile_pool(name="ps23", bufs=1, space="PSUM") as ps23,
            tc.tile_pool(name="psc", bufs=2, space="PSUM") as psc,
            tc.tile_pool(name="ps4", bufs=1, space="PSUM") as ps4,
        ):
            # ---- static weights (reg first: it gates the dispatch) ----
            reg_sb = constp.tile([1, NB], mybir.dt.int32)
            nc.sync.dma_start(reg_sb[:], reg_d[:])
            ai_sb = constp.tile([D, D], BF16)
            nc.sync.dma_start(ai_sb[:], ai_d[:])
            wc_sb = constp.tile([D, 2 * H2], BF16)   # [[Wc;0] | [0;Wc]]
            nc.sync.dma_start(wc_sb[:], wc_d[:])
            wo_sb = constp.tile([H2, 32], BF16)
            nc.sync.dma_start(wo_sb[:], wo_d[:])
            bc_sb = constp.tile([H2, 1], F32)
            nc.sync.dma_start(bc_sb[:], bc_d[:])

            # ---- batch-0 first chunks via the idle ACT ring: fast start
            zz_t, zc_t = [], []
            for b in range(NB):
                zz_t.append(zinp.tile([D, T * L], BF16, tag="zz",
                                      name=f"zz{b}"))
                zc_t.append(zinp.tile([D, T * L], BF16, tag="zc",
                                      name=f"zc{b}"))
            nc.scalar.dma_start(zz_t[0][:, 0:2048], zs_d[0, :, 0:2048])
            nc.scalar.dma_start(zc_t[0][:, 0:2048], zc_d[0, :, 0:2048])

            # ---- per-batch dispatched weights (regime -> env) ----
            w1s_sb, b1s_sb, w2_sb, b2b_sb = [], [], [], []
            for b in range(NB):
                e = nc.values_load(
                    reg_sb[0:1, b : b + 1],
                    engines=[mybir.EngineType.SP],
                    min_val=0, max_val=NE - 1,
                    skip_runtime_bounds_check=True,
                )
                w1 = constp.tile([D, H], F32R, name=f"w1s{b}", tag=f"w1s{b}")
                nc.sync.dma_start(
                    w1[:], w1s_d[bass.ds(e, 1)].rearrange("o p h -> (o p) h")
                )
                b1 = constp.tile([D, 2], F32, name=f"b1s{b}", tag=f"b1s{b}")
                nc.sync.dma_start(
                    b1[:], b1s_d[bass.ds(e, 1)].rearrange("o p h -> (o p) h")
                )
                w2 = constp.tile([D, 2, 32], BF16, name=f"w2{b}", tag=f"w2{b}")
                nc.sync.dma_start(
                    w2[:], w2p_d[bass.ds(e, 1)].rearrange("o p a k -> (o p) a k")
                )
                b2 = constp.tile([D, 1], F32, name=f"b2b{b}", tag=f"b2b{b}")
                nc.sync.dma_start(
                    b2[:], b2b_d[bass.ds(e, 1)].rearrange("o p h -> (o p) h")
                )
                w1s_sb.append(w1)
                b1s_sb.append(b1)
                w2_sb.append(w2)
                b2b_sb.append(b2)

            # ---- batch-0 second halves on sync (needed from oct 4) ----
            nc.sync.dma_start(zz_t[0][:, 2048:4096], zs_d[0, :, 2048:4096])
            nc.sync.dma_start(zc_t[0][:, 2048:4096], zc_d[0, :, 2048:4096])

            st_sig = finp.tile([NB * 32, 512], F32)

            # preload the Ln activation table so the per-batch softplus
            # doesn't pay a 1.3us ACT_TABLE_LOAD on the critical tail
            warm = finp.tile([1, 1], F32)
            nc.scalar.activation(warm[:], bc_sb[0:1, 0:1], AF.Ln, bias=1.0)
            nc.scalar.activation(warm[:], warm[:], AF.Exp)

            for b in range(NB):
                zz = zz_t[b]
                zc = zc_t[b]

                if b + 1 < NB:
                    # prefetch next batch's inputs (distance 1 keeps the
                    # sync ring clear for this batch's output DMAs)
                    nc.sync.dma_start(zz_t[b + 1][:], zs_d[b + 1])
                    nc.sync.dma_start(zc_t[b + 1][:], zc_d[b + 1])

                for o in range(8):
                    # ---- stage 1: 4 signal pair matmuls, N=128 ----
                    p1 = ps1.tile([D, 512], F32, tag="p1")
                    for qt in range(4):           # qt = qq*2+tp
                        pr = o * 4 + qt
                        nc.tensor.matmul(
                            p1[:, 128 * qt : 128 * (qt + 1)],
                            zz[:, 128 * pr : 128 * (pr + 1)],
                            ai_sb[:],
                            start=True, stop=True,
                        )
                    # ---- C1 (independent of zzt; hides the cast) ----
                    # pc_q[h2, (tq,i)] per qq; par tiles run concurrently
                    pcs = []
                    for qq in range(2):
                        pc = psc.tile([D, 512], F32, tag="pc")
                        for par in range(2):
                            nc.tensor.matmul(
                                pc[:, 256 * par : 256 * (par + 1)],
                                wc_sb[:, 128 * par : 128 * (par + 1)],
                                zc[:, 512 * o + 256 * qq :
                                   512 * o + 256 * qq + 256],
                                start=True, stop=True,
                            )
                        pcs.append(pc)

                    # ---- stage-1 evacuation: fp32 PSUM -> bf16 SBUF ----
                    zzt = evp.tile([D, 512], F32R, tag="zzt")
                    nc.vector.tensor_copy(zzt[:], p1[:])


                    # ---- S23 (hh-major) + h1 evac right after each hh ----
                    # h1 cols: hh*1024 + qq*512 + tq*128 + i  (tq = 2*par+tp)
                    h1 = evp.tile([D, 2048], BF16, tag="h1")
                    h1v = h1[:].rearrange(
                        "p (hh qq par c) -> p hh qq par c", hh=2, qq=2, par=2)
                    for hh in range(2):
                        ph = ps23.tile([D, 1024], F32, tag=f"p23h{hh}")
                        for par in range(2):
                            nc.tensor.matmul(
                                ph[:, 512 * par : 512 * par + 512],
                                w1s_sb[b][64 * par : 64 * par + 64,
                                          128 * hh : 128 * (hh + 1)],
                                zzt[64 * par : 64 * par + 64, :],
                                start=True, stop=True,
                            )
                        in_ap = ph[:].rearrange(
                            "p (par qq c) -> p par qq c", par=2, qq=2
                        ).transpose([0, 2, 1, 3])
                        out_ap = h1v[:, hh]
                        if hh == 0:
                            nc.scalar.activation(
                                out_ap, in_ap, AF.Relu,
                                bias=b1s_sb[b][:, 0:1],
                            )
                        else:
                            nc.vector.tensor_scalar(
                                out_ap, in_ap,
                                b1s_sb[b][:, 1:2], 0.0, ADD, MAX,
                            )


                    # ---- hc evac: relu(pc + bc) -> bf16 ----
                    hcs = evp.tile([D, 1024], BF16, tag="hcs")
                    nc.scalar.activation(
                        hcs[:, 0:512], pcs[0][:], AF.Relu, bias=bc_sb[:, 0:1])
                    nc.scalar.activation(
                        hcs[:, 512:768], pcs[1][:, 0:256], AF.Relu,
                        bias=bc_sb[:, 0:1])
                    nc.vector.tensor_scalar(
                        hcs[:, 768:1024], pcs[1][:, 256:512],
                        bc_sb[:, 0:1], 0.0, ADD, MAX)

                    # ---- S4 + C2: col-packed quads, 2 per PSUM bank ----
                    p4 = ps4.tile([D, 512], F32, tag="p4")
                    for qq in range(2):
                        bp = qq * 32
                        nc.tensor.matmul(
                            p4[bp : bp + 32, :], w2_sb[b][:, 0, :],
                            h1[:, 512 * qq : 512 * qq + 512],
                            start=True, stop=False,
                        )
                        nc.tensor.matmul(
                            p4[bp : bp + 32, :], w2_sb[b][:, 1, :],
                            h1[:, 1024 + 512 * qq : 1024 + 512 * qq + 512],
                            start=False, stop=False,
                        )
                        nc.tensor.matmul(
                            p4[bp : bp + 32, :], wo_sb[:],
                            hcs[:, 512 * qq : 512 * qq + 512],
                            start=False, stop=True,
                        )
                    # one bias pass covers both quads (rows 0,1,32,33)
                    if o % 2 == 0:
                        stb2 = stgp.tile([64, 1024], F32, tag="stb")
                    nc.scalar.activation(
                        stb2[:, 512 * (o % 2) : 512 * (o % 2) + 512],
                        p4[0:64, :], AF.Identity, bias=b2b_sb[b][0:64, 0:1])
                    if o % 2 == 1:
                        o0 = o - 1
                        for q in range(2):
                            off = 1024 * o0 + 512 * q
                            nc.sync.dma_start(
                                mu_d[b : b + 1, off : off + 1536]
                                .rearrange("z (h c) -> z h c", h=3)[:, 0::2],
                                stb2[32 * q : 32 * q + 1, :],
                            )
                            r0 = b * 32 + 8 * q + o0
                            nc.sync.dma_start(
                                st_sig[r0 : r0 + 2, :],
                                stb2[32 * q + 1 : 32 * q + 2, :],
                            )

                # ---- sigma tail for this batch: softplus + 0.01 + out ----
                sgrows = st_sig[b * 32 : b * 32 + 16, :]
                ex = stgp.tile([16, 512], F32, tag="ex")
                nc.scalar.activation(ex[:], sgrows, AF.Exp)
                nc.scalar.activation(sgrows, ex[:], AF.Ln, bias=1.0)
                nc.vector.tensor_scalar_add(sgrows, sgrows, 0.01)
                for q in range(2):
                    nc.sync.dma_start(
                        sg_d[b : b + 1, :]
                        .rearrange("z (o q c) -> z o q c", o=8, q=2)[:, :, q],
                        st_sig[b * 32 + 8 * q : b * 32 + 8 * q + 8, :],
                    )



    nc.compile()
    return nc


def _get_nc():
    if "nc" not in _CACHE:
        _CACHE["nc"] = _build()
    return _CACHE["nc"]


def _prepare_in_maps(z_signal, z_corrupt, A, regime, W_sig, b_sig, W1e, b1e,
                     W2e, b2e, Wc, bc, Wo, bo):
    bf16 = ml_dtypes.bfloat16
    z_signal = np.asarray(z_signal, dtype=np.float32)
    z_corrupt = np.asarray(z_corrupt, dtype=np.float32)
    A = np.asarray(A, dtype=np.float32)
    regime = np.asarray(regime)
    W_sig = np.asarray(W_sig, dtype=np.float32)
    b_sig = np.asarray(b_sig, dtype=np.float32)
    W1e = np.asarray(W1e, dtype=np.float32)
    b1e = np.asarray(b1e, dtype=np.float32)
    W2e = np.asarray(W2e, dtype=np.float32)
    b2e = np.asarray(b2e, dtype=np.float32)
    Wc = np.asarray(Wc, dtype=np.float32)
    bc = np.asarray(bc, dtype=np.float32)
    Wo = np.asarray(Wo, dtype=np.float32)
    bo = np.asarray(bo, dtype=np.float32)

    eidx = np.where(regime >= NE, 0, regime).astype(np.int32)

    # ---- host weight transforms (env tables, replicated to all cores) ----
    ai = A.astype(bf16)
    w1s_half = np.einsum("lh,ehk->elk", W_sig, W1e)            # [E, L, H]
    w1s = _round_fp32r(
        np.ascontiguousarray(np.concatenate([w1s_half, w1s_half], axis=1)))
    b1s_full = np.einsum("h,ehk->ek", b_sig, W1e) + b1e        # [E, H]
    b1s = np.ascontiguousarray(
        b1s_full.reshape(NE, 2, D).transpose(0, 2, 1))         # [E, D, 2]
    w2p = np.zeros((NE, D, 2, 32), np.float32)
    w2p[..., 0:2] = W2e.reshape(NE, 2, D, 2).transpose(0, 2, 1, 3)
    w2p = w2p.astype(bf16)
    b2b = np.zeros((NE, D, 1), np.float32)
    b2b[:, 0::32, 0] = (b2e[:, 0] + bo[0])[:, None]
    b2b[:, 1::32, 0] = b2e[:, 1][:, None]
    wc_r = np.zeros((D, 2 * H2), np.float32)                   # [[Wc;0]|[0;Wc]]
    wc_r[0:64, 0:H2] = Wc
    wc_r[64:128, H2:] = Wc
    wc_r = wc_r.astype(bf16)
    wo_r = np.zeros((H2, 32), np.float32)
    wo_r[:, 0:1] = Wo
    wo_r = wo_r.astype(bf16)
    bc_r = np.ascontiguousarray(bc[:, None])                   # [H2, 1]

    in_maps = []
    for c in range(N_CORES):
        b0 = c * NB
        zs4 = z_signal[b0 : b0 + NB]
        zc4 = z_corrupt[b0 : b0 + NB]
        # signal: [nb, D, (o,qq,tp), (t01,l)] — pair pr holds (t, t+2)
        zt = zs4.transpose(0, 2, 1, 3).reshape(NB, D, 8, 2, 2, 2, L)
        zs_p = np.ascontiguousarray(
            zt.transpose(0, 1, 2, 3, 5, 4, 6).reshape(NB, D, T * L)
        ).astype(bf16)
        # corrupt (host-transposed): [nb, (t01,l), (o,qq,tp,i)]
        zcr = zc4.reshape(NB, 8, 2, 2, 2, D, L)
        zc_p = np.ascontiguousarray(
            zcr.transpose(0, 3, 6, 1, 2, 4, 5).reshape(NB, D, T * L)
        ).astype(bf16)
        in_maps.append({
            "zs": zs_p,
            "zc": zc_p,
            "ai": ai,
            "reg": eidx[None, b0 : b0 + NB],
            "w1s": w1s,
            "b1s": b1s,
            "w2p": w2p,
            "b2b": b2b,
            "wc": wc_r,
            "bc": bc_r,
            "wo": wo_r,
        })
    return in_maps


def kernel(z_signal, z_corrupt, A, regime, W_sig, b_sig, W1e, b1e, W2e, b2e,
           Wc, bc, Wo, bo):
    from concourse.bass_utils import run_bass_kernel_spmd

    in_maps = _prepare_in_maps(z_signal, z_corrupt, A, regime, W_sig, b_sig,
                               W1e, b1e, W2e, b2e, Wc, bc, Wo, bo)
    nc = _get_nc()
    res = run_bass_kernel_spmd(nc, in_maps, core_ids=list(range(N_CORES)))

    mu = np.concatenate(
        [r["mu"].reshape(NB, T, D) for r in res.results], axis=0)
    sigma = np.concatenate(
        [r["sg"].reshape(NB, T, D) for r in res.results], axis=0)
    return mu, sigma


def run_traced(inputs_np):
    from concourse.bass_utils import run_bass_kernel_spmd

    in_maps = _prepare_in_maps(**inputs_np)
    nc = _get_nc()
    return run_bass_kernel_spmd(
        nc, in_maps, core_ids=list(range(N_CORES)), trace=True
    )
